# revision 1
# baseline (speedup 1.0000x reference)
"""BiRGAT (bipartite 2-layer GATv2) Trainium2 kernel, 8-core SPMD.

Strategy: destination-tile sharding. Gene dsts padded to 160 tiles of 128
(20 tiles/core), sample dsts 32 tiles (4/core), dealt to cores by sorted
chunk count so every core runs an identical baked per-slot chunk schedule.
Edge-phase per 128-edge chunk: indirect-DMA gather of source rows, GATv2
attention (Prelu + fused dot via scalar_tensor_tensor accum_out, exp),
one-hot matmul scatter-add of [messages | ea] into PSUM. Tile evacuation
does softmax normalization (no segment_max: alpha std ~0.3; the reference's
den+1e-16 makes max-subtraction irrelevant), bias, ELU, residuals.
Source-side tables are AllGathered between phases (overlapped with edge
compute by the Tile scheduler).
"""
import sys

sys.path.insert(0, "/opt/trn_rl_repo")

import numpy as np
from contextlib import ExitStack

import concourse.bass as bass
import concourse.tile as tile
from concourse import bacc, mybir
from concourse.bass_utils import run_bass_kernel_spmd
from concourse.masks import make_identity

P = 128
NCORES = 8
NS, NG, E = 4096, 20000, 131072
DIN, H, C1, C3 = 256, 4, 64, 128
HC1, HC3 = H * C1, H * C3          # 256, 512
NGP = 20480                        # genes padded to 160 tiles
NST, NGT = NS // P, NGP // P       # 32, 160
S_PER_CORE, G_PER_CORE = NST // NCORES, NGT // NCORES   # 4, 20
SROWS, GROWS = S_PER_CORE * P, G_PER_CORE * P           # 512, 2560

F32 = mybir.dt.float32
I32 = mybir.dt.int32
AF = mybir.ActivationFunctionType
OP = mybir.AluOpType

PAD_LOC = 200.0   # dst-local sentinel for padded edges (never equals 0..127)


# ---------------------------------------------------------------- host plan

def _deal_tiles(dst, n_tiles, per_core):
    """Deal dst tiles to cores by sorted chunk count. Returns
    assign[slot, core] -> tile id, sched[slot] -> chunks, per-tile counts."""
    tcnt = np.bincount(dst // P, minlength=n_tiles)
    chunks = np.maximum((tcnt + P - 1) // P, 1)
    order = np.argsort(-chunks, kind="stable")
    assign = order.reshape(per_core, NCORES)
    sched = chunks[assign].max(axis=1)
    return assign, sched.astype(int), tcnt


def _edge_arrays(src, dst, assign, sched, src_row_map, core):
    """Per-core edge chunk arrays for one relation.
    Returns src_rows [P, NCH] i32, dstrow [P, NCH] i32, dstloc [P, NCH] f32."""
    nch = int(sched.sum())
    src_rows = np.zeros((nch, P), np.int32)
    dstrow = np.zeros((nch, P), np.int32)
    dstloc = np.full((nch, P), PAD_LOC, np.float32)
    tile_of = dst // P
    ci = 0
    for slot in range(len(sched)):
        t = assign[slot, core]
        e = np.nonzero(tile_of == t)[0]
        n = len(e)
        want = sched[slot] * P
        s = np.zeros(want, np.int32)
        dl = np.full(want, PAD_LOC, np.float32)
        dr = np.zeros(want, np.int32)
        s[:n] = src_row_map[src[e]]
        dl[:n] = (dst[e] % P).astype(np.float32)
        dr[:n] = slot * P + dst[e] % P
        src_rows[ci:ci + sched[slot]] = s.reshape(-1, P)
        dstloc[ci:ci + sched[slot]] = dl.reshape(-1, P)
        dstrow[ci:ci + sched[slot]] = dr.reshape(-1, P)
        ci += sched[slot]
    return src_rows.T.copy(), dstrow.T.copy(), dstloc.T.copy()


def _bcast(v, p=P):
    return np.broadcast_to(np.asarray(v, np.float32).reshape(1, -1),
                           (p, len(np.asarray(v).reshape(-1)))).copy()


def _plan(inputs):
    sg_src = np.asarray(inputs["sg_src"]); sg_dst = np.asarray(inputs["sg_dst"])
    gs_src = np.asarray(inputs["gs_src"]); gs_dst = np.asarray(inputs["gs_dst"])

    g_assign, g_sched, _ = _deal_tiles(sg_dst, NGT, G_PER_CORE)
    s_assign, s_sched, _ = _deal_tiles(gs_dst, NST, S_PER_CORE)

    # tile -> (owner core, slot)
    g_owner = np.zeros(NGT, np.int32); g_slot = np.zeros(NGT, np.int32)
    for slot in range(G_PER_CORE):
        for c in range(NCORES):
            g_owner[g_assign[slot, c]] = c
            g_slot[g_assign[slot, c]] = slot
    s_owner = np.zeros(NST, np.int32); s_slot = np.zeros(NST, np.int32)
    for slot in range(S_PER_CORE):
        for c in range(NCORES):
            s_owner[s_assign[slot, c]] = c
            s_slot[s_assign[slot, c]] = slot

    sid = np.arange(NS)
    srow_tbl = s_owner[sid // P] * SROWS + s_slot[sid // P] * P + sid % P
    gid = np.arange(NG)
    grow_tbl = g_owner[gid // P] * GROWS + g_slot[gid // P] * P + gid % P

    plan = {
        "g_assign": g_assign, "g_sched": g_sched,
        "s_assign": s_assign, "s_sched": s_sched,
        "srow_tbl": srow_tbl, "grow_tbl": grow_tbl,
    }

    x_sample = np.asarray(inputs["x_sample"], np.float32)
    x_gene = np.asarray(inputs["x_gene"], np.float32)

    in_maps = []
    for c in range(NCORES):
        # node rows owned by this core, in slot order
        s_tiles = s_assign[:, c]
        xs_own = x_sample.reshape(NST, P, DIN)[s_tiles].reshape(SROWS, DIN)
        g_tiles = g_assign[:, c]
        xg_own = np.zeros((GROWS, DIN), np.float32)
        for i, t in enumerate(g_tiles):
            lo = t * P
            if lo < NG:
                n = min(P, NG - lo)
                xg_own[i * P:i * P + n] = x_gene[lo:lo + n]

        sgS, sgR, sgL = _edge_arrays(sg_src, sg_dst, g_assign, g_sched,
                                     srow_tbl, c)
        gsS, gsR, gsL = _edge_arrays(gs_src, gs_dst, s_assign, s_sched,
                                     grow_tbl, c)
        gsS3 = _edge_arrays(gs_src, gs_dst, s_assign, s_sched,
                            grow_tbl, c)[0]  # same rows; tbl3 shares layout

        m = {
            "xs_own": np.ascontiguousarray(xs_own),
            "xg_own": np.ascontiguousarray(xg_own),
            "Wl1_sg": np.asarray(inputs["Wl1_sg"], np.float32),
            "Wr1_sg": np.asarray(inputs["Wr1_sg"], np.float32),
            "Wl1_gs": np.asarray(inputs["Wl1_gs"], np.float32),
            "Wr1_gs": np.asarray(inputs["Wr1_gs"], np.float32),
            "Wl3": np.asarray(inputs["Wl3_gs"], np.float32),
            "Wr3": np.asarray(inputs["Wr3_gs"], np.float32),
            "sl1_W": np.asarray(inputs["sl1_W"], np.float32),
            "sl3_W": np.asarray(inputs["sl3_W"], np.float32),
            "att1_sg_b": _bcast(np.asarray(inputs["att1_sg"]).reshape(-1)),
            "att1_gs_b": _bcast(np.asarray(inputs["att1_gs"]).reshape(-1)),
            "att3_b": _bcast(np.asarray(inputs["att3_gs"]).reshape(-1)),
            "bl1_sg_b": _bcast(inputs["bl1_sg"]),
            "br1_sg_b": _bcast(inputs["br1_sg"]),
            "bl1_gs_b": _bcast(inputs["bl1_gs"]),
            "br1_gs_b": _bcast(inputs["br1_gs"]),
            "bias1_sg_b": _bcast(inputs["bias1_sg"]),
            "bias1_gs_b": _bcast(inputs["bias1_gs"]),
            "bl3_b": _bcast(inputs["bl3_gs"]),
            "br3_b": _bcast(inputs["br3_gs"]),
            "bias3_b": _bcast(inputs["bias3_gs"]),
            "sl1_b_b": _bcast(inputs["sl1_b"]),
            "sl3_b_b": _bcast(inputs["sl3_b"]),
            "sg_srcr": sgS, "sg_dstl": sgL,
            "gs_srcr": gsS, "gs_dstl": gsL,
            "gs_srcr3": gsS3,
        }
        in_maps.append(m)
    return plan, in_maps


# ------------------------------------------------------------- device build

def _load_w(nc, pool, w_dram, kdim, n, tag):
    """Load [kdim, n] weight into SBUF as [128, kdim//128, n] rhs tiles."""
    kc = kdim // P
    t = pool.tile([P, kc, n], F32, tag=tag)
    nc.sync.dma_start(t[:], w_dram[:].rearrange("(c p) n -> p c n", p=P))
    return t


def _transpose2(nc, sb, psp, ident, x_ap, kc):
    """PE-transpose x [128, kc*128] -> list of kc SBUF tiles [128,128]."""
    outs = []
    for k in range(kc):
        pt = psp.tile([P, P], F32, space="PSUM", tag="transp")
        nc.tensor.transpose(out=pt[:], in_=x_ap[:, k * P:(k + 1) * P],
                            identity=ident[:])
        st = sb.tile([P, P], F32, tag="transs")
        nc.scalar.copy(st[:], pt[:])
        outs.append(st)
    return outs


F32R = mybir.dt.float32r


def _r(ap):
    """Matmul operand passthrough (fp32r rejected by walrus: producers
    must pre-round; revisit if PE becomes the bottleneck)."""
    return ap


def _ap3(base_ap, h, c, mid, inner):
    """[128, h, c] view over base_ap's tensor with given free strides."""
    return bass.AP(base_ap.tensor, base_ap.offset,
                   [[base_ap.ap[0][0], P], [mid, h], [inner, c]])


def _mm_kc(nc, psum_ap, xT, w_sb, n):
    kc = len(xT)
    for k in range(kc):
        nc.tensor.matmul(psum_ap, lhsT=_r(xT[k][:]), rhs=_r(w_sb[:, k, :n]),
                         start=(k == 0), stop=(k == kc - 1))


def _elu(nc, sb, out_ap, y_ap, w):
    """out = elu(y) = (relu(y) - 1) + exp(min(y, 0)); [128, w] tiles."""
    m = sb.tile([P, w], F32, tag="elu_m")
    nc.vector.tensor_scalar(out=m[:], in0=y_ap, scalar1=0.0, scalar2=None,
                            op0=OP.min)
    e = sb.tile([P, w], F32, tag="elu_e")
    nc.scalar.activation(e[:], m[:], AF.Exp)
    r = sb.tile([P, w], F32, tag="elu_r")
    nc.scalar.activation(r[:], y_ap, AF.Relu)
    nc.vector.scalar_tensor_tensor(out=out_ap, in0=r[:], scalar=-1.0,
                                   in1=e[:], op0=OP.add, op1=OP.add)


def _build(g_sched, s_sched):
    nsg = int(g_sched.sum())
    ngs = int(s_sched.sum())
    nc = bacc.Bacc("TRN2", target_bir_lowering=False, debug=False,
                   num_devices=NCORES)

    ei = lambda name, shape, dt=F32: nc.dram_tensor(name, shape, dt,
                                                    kind="ExternalInput")
    xs_own = ei("xs_own", [SROWS, DIN]); xg_own = ei("xg_own", [GROWS, DIN])
    Wl1_sg = ei("Wl1_sg", [DIN, HC1]); Wr1_sg = ei("Wr1_sg", [DIN, HC1])
    Wl1_gs = ei("Wl1_gs", [DIN, HC1]); Wr1_gs = ei("Wr1_gs", [DIN, HC1])
    Wl3 = ei("Wl3", [HC1, HC3]); Wr3 = ei("Wr3", [HC1, HC3])
    sl1_W = ei("sl1_W", [DIN, C1]); sl3_W = ei("sl3_W", [HC1, C3])
    att1_sg_b = ei("att1_sg_b", [P, HC1]); att1_gs_b = ei("att1_gs_b", [P, HC1])
    att3_b = ei("att3_b", [P, HC3])
    bl1_sg_b = ei("bl1_sg_b", [P, HC1]); br1_sg_b = ei("br1_sg_b", [P, HC1])
    bl1_gs_b = ei("bl1_gs_b", [P, HC1]); br1_gs_b = ei("br1_gs_b", [P, HC1])
    bias1_sg_b = ei("bias1_sg_b", [P, HC1]); bias1_gs_b = ei("bias1_gs_b", [P, HC1])
    bl3_b = ei("bl3_b", [P, HC3]); br3_b = ei("br3_b", [P, HC3])
    bias3_b = ei("bias3_b", [P, C3])
    sl1_b_b = ei("sl1_b_b", [P, C1]); sl3_b_b = ei("sl3_b_b", [P, C3])
    sg_srcr = ei("sg_srcr", [P, nsg], I32)
    sg_dstl = ei("sg_dstl", [P, nsg]); gs_srcr = ei("gs_srcr", [P, ngs], I32)
    gs_dstl = ei("gs_dstl", [P, ngs])
    gs_srcr3 = ei("gs_srcr3", [P, ngs], I32)

    out_own = nc.dram_tensor("out_own", [SROWS, C3], F32, kind="ExternalOutput")

    # DRAM scratch
    agin_s = nc.dram_tensor("agin_s", [SROWS, HC1], F32R)
    agin_g = nc.dram_tensor("agin_g", [GROWS, HC1], F32R)
    agin_3 = nc.dram_tensor("agin_3", [GROWS, HC3], F32R)
    tbl_s = nc.dram_tensor("tbl_s", [NS, HC1], F32R, addr_space="Shared")
    tbl_g = nc.dram_tensor("tbl_g", [NGP, HC1], F32R, addr_space="Shared")
    tbl_3 = nc.dram_tensor("tbl_3", [NGP, HC3], F32R, addr_space="Shared")
    xr1_sg = nc.dram_tensor("xr1_sg", [GROWS, HC1], F32)
    xr1_gs = nc.dram_tensor("xr1_gs", [SROWS, HC1], F32)
    xr3 = nc.dram_tensor("xr3", [SROWS, HC3], F32)

    RG = [list(range(NCORES))]

    with tile.TileContext(nc) as tc, ExitStack() as ctx:
        res = ctx.enter_context(tc.tile_pool(name="res", bufs=1))
        wp = ctx.enter_context(tc.tile_pool(name="wp", bufs=1))
        sb = ctx.enter_context(tc.tile_pool(name="sb", bufs=6))
        ev = ctx.enter_context(tc.tile_pool(name="ev", bufs=2))
        psp = ctx.enter_context(tc.tile_pool(name="psp", bufs=2, space="PSUM"))
        ps1 = ctx.enter_context(tc.tile_pool(name="ps1", bufs=1, space="PSUM"))
        pse = ctx.enter_context(tc.tile_pool(name="pse", bufs=1, space="PSUM"))
        psx = ctx.enter_context(tc.tile_pool(name="psx", bufs=4, space="PSUM"))

        ident = res.tile([P, P], F32)
        make_identity(nc, ident[:])
        ident_r = res.tile([P, P], F32R)
        nc.scalar.copy(ident_r[:], ident[:])
        iota = res.tile([P, P], F32)
        nc.gpsimd.iota(iota[:], pattern=[[1, P]], base=0, channel_multiplier=0,
                       allow_small_or_imprecise_dtypes=True)

        def rload(name, dram, shape, dt=F32):
            t = res.tile(shape, dt, tag=name)
            nc.sync.dma_start(t[:], dram[:])
            return t

        att1_sg_t = rload("a1s", att1_sg_b, [P, HC1])
        att1_gs_t = rload("a1g", att1_gs_b, [P, HC1])
        att3_t = rload("a3", att3_b, [P, HC3])
        bias1_sg_t = rload("b1s", bias1_sg_b, [P, HC1])
        bias1_gs_t = rload("b1g", bias1_gs_b, [P, HC1])
        bias3_t = rload("b3", bias3_b, [P, C3])
        sg_srcr_t = rload("sgs", sg_srcr, [P, nsg], I32)
        sg_dstl_t = rload("sgl", sg_dstl, [P, nsg])
        gs_srcr_t = rload("gss", gs_srcr, [P, ngs], I32)
        gs_dstl_t = rload("gsl", gs_dstl, [P, ngs])
        gs_srcr3_t = rload("gs3", gs_srcr3, [P, ngs], I32)

        sl1_sb = res.tile([P, S_PER_CORE * C1], F32)   # sl1 rows per slot
        sl3_sb = res.tile([P, S_PER_CORE * C3], F32)   # sl3 rows per slot

        # weights (rhs layout [128, kc, n])
        Wl1_sg_t = _load_w(nc, wp, Wl1_sg, DIN, HC1, "Wl1_sg")
        Wr1_sg_t = _load_w(nc, wp, Wr1_sg, DIN, HC1, "Wr1_sg")
        Wl1_gs_t = _load_w(nc, wp, Wl1_gs, DIN, HC1, "Wl1_gs")
        Wr1_gs_t = _load_w(nc, wp, Wr1_gs, DIN, HC1, "Wr1_gs")
        Wl3_t = _load_w(nc, wp, Wl3, HC1, HC3, "Wl3")
        Wr3_t = _load_w(nc, wp, Wr3, HC1, HC3, "Wr3")
        sl1_W_t = _load_w(nc, wp, sl1_W, DIN, C1, "sl1_W")
        sl3_W_t = _load_w(nc, wp, sl3_W, HC1, C3, "sl3_W")
        bl1_sg_t = rload("bl1s", bl1_sg_b, [P, HC1])
        br1_sg_t = rload("br1s", br1_sg_b, [P, HC1])
        bl1_gs_t = rload("bl1g", bl1_gs_b, [P, HC1])
        br1_gs_t = rload("br1g", br1_gs_b, [P, HC1])
        bl3_t = rload("bl3", bl3_b, [P, HC3])
        br3_t = rload("br3", br3_b, [P, HC3])
        sl1_b_t = rload("sl1b", sl1_b_b, [P, C1])
        sl3_b_t = rload("sl3b", sl3_b_b, [P, C3])

        def dense_out(xT, w_sb, n, bias_t, dst_ap=None, sbuf_dst=None,
                      rdt=F32):
            pt = ps1.tile([P, n], F32, space="PSUM", tag="aux")
            _mm_kc(nc, pt[:], xT, w_sb, n)
            o = sbuf_dst if sbuf_dst is not None else sb.tile([P, n], rdt,
                                                             tag="dout")
            nc.vector.tensor_tensor(out=o[:] if sbuf_dst is None else sbuf_dst,
                                    in0=pt[:, :n], in1=bias_t[:, :n],
                                    op=OP.add)
            if dst_ap is not None:
                nc.sync.dma_start(dst_ap, o[:])
            return o

        # ---- phase A: sample node tables, AG1a
        for i in range(S_PER_CORE):
            xs = sb.tile([P, DIN], F32, tag="xnode")
            nc.sync.dma_start(xs[:], xs_own[i * P:(i + 1) * P, :])
            xT = _transpose2(nc, sb, psp, ident, xs[:], DIN // P)
            dense_out(xT, Wl1_sg_t, HC1, bl1_sg_t,
                      dst_ap=agin_s[i * P:(i + 1) * P, :], rdt=F32R)
            dense_out(xT, Wr1_gs_t, HC1, br1_gs_t,
                      dst_ap=xr1_gs[i * P:(i + 1) * P, :])
            dense_out(xT, sl1_W_t, C1, sl1_b_t,
                      sbuf_dst=sl1_sb[:, i * C1:(i + 1) * C1])
        nc.gpsimd.collective_compute("AllGather", OP.bypass, replica_groups=RG,
                                     ins=[agin_s[:]], outs=[tbl_s[:]])

        # ---- phase A: gene node tables, AG1b
        for j in range(G_PER_CORE):
            xg = sb.tile([P, DIN], F32, tag="xnode")
            nc.sync.dma_start(xg[:], xg_own[j * P:(j + 1) * P, :])
            xT = _transpose2(nc, sb, psp, ident, xg[:], DIN // P)
            dense_out(xT, Wl1_gs_t, HC1, bl1_gs_t,
                      dst_ap=agin_g[j * P:(j + 1) * P, :], rdt=F32R)
            dense_out(xT, Wr1_sg_t, HC1, br1_sg_t,
                      dst_ap=xr1_sg[j * P:(j + 1) * P, :])
        nc.gpsimd.collective_compute("AllGather", OP.bypass, replica_groups=RG,
                                     ins=[agin_g[:]], outs=[tbl_g[:]])

        # ---- edge chunk body
        def edge_chunk(ci, srcr_t, dstl_t, tbl, xr_slot, att_t,
                       psum_m, psum_d, first, last, w):
            xl = sb.tile([P, w], F32R, tag=f"xl{w}")
            nc.gpsimd.indirect_dma_start(
                out=xl[:], out_offset=None, in_=tbl[:],
                in_offset=bass.IndirectOffsetOnAxis(ap=srcr_t[:, ci:ci + 1],
                                                    axis=0))
            # one-hot of dst-local index (also kills padded edges)
            onehot = sb.tile([P, P], F32R, tag="onehot")
            nc.vector.tensor_scalar(out=onehot[:], in0=iota[:],
                                    scalar1=dstl_t[:, ci:ci + 1], scalar2=None,
                                    op0=OP.is_equal)
            # xr[e,:] = xr_slot[dstloc_e,:] via ohT.T @ xr_slot on PE
            ohTp = psp.tile([P, P], F32R, space="PSUM", tag="transp")
            nc.tensor.transpose(out=ohTp[:], in_=onehot[:], identity=ident_r[:])
            ohT = sb.tile([P, P], F32R, tag="ohT")
            nc.scalar.copy(ohT[:], ohTp[:])
            xrg = psx.tile([P, HC3], F32, space="PSUM", tag="xrg")
            nc.tensor.matmul(xrg[:, :w], lhsT=ident_r[:], rhs=xl[:],
                             start=True, stop=False)
            nc.tensor.matmul(xrg[:, :w], lhsT=ohT[:],
                             rhs=xr_slot[:, :w], start=False, stop=True)
            g = sb.tile([P, w], F32, tag=f"g{w}")
            nc.scalar.activation(g[:], xrg[:, :w], AF.Prelu, alpha=0.2)
            ch = w // H
            alpha4 = sb.tile([P, H], F32, tag="alpha4")
            junk = sb.tile([P, w // H], F32, tag="junk")
            for h in range(H):
                sl = slice(h * ch, (h + 1) * ch)
                nc.vector.scalar_tensor_tensor(
                    out=junk[:, :ch], in0=g[:, sl], scalar=1.0,
                    in1=att_t[:, sl], op0=OP.mult, op1=OP.mult,
                    accum_out=alpha4[:, h:h + 1])
            msgs = sb.tile([P, w + H], F32R, tag=f"msgs{w}")
            nc.scalar.activation(msgs[:, w:w + H], alpha4[:], AF.Exp)
            # msgs[:, :w] = xl * ea (per-head broadcast) in one 3D TT
            nc.vector.tensor_tensor(
                out=_ap3(msgs[:], H, ch, ch, 1),
                in0=_ap3(xl[:].bitcast(F32), H, ch, ch, 1),
                in1=_ap3(msgs[:, w:w + H], H, ch, 1, 0),
                op=OP.mult)
            if w == HC1:
                nc.tensor.matmul(psum_m[:], lhsT=onehot[:],
                                 rhs=msgs[:, :w + H], start=first,
                                 stop=last)
            else:
                nc.tensor.matmul(psum_m[:], lhsT=onehot[:],
                                 rhs=msgs[:, :w], start=first, stop=last)
                nc.tensor.matmul(psum_d[:], lhsT=onehot[:],
                                 rhs=msgs[:, w:w + H], start=first,
                                 stop=last)

        def norm_heads(psum_ap, den_ap, w, tag):
            """y[:, h*ch:(h+1)*ch] = psum_h / (den_h + 1e-16)."""
            ch = w // H
            den = sb.tile([P, H], F32, tag="den")
            nc.vector.tensor_scalar(out=den[:], in0=den_ap, scalar1=1e-16,
                                    scalar2=None, op0=OP.add)
            rden = sb.tile([P, H], F32, tag="rden")
            nc.vector.reciprocal(rden[:], den[:])
            y = ev.tile([P, w], F32, tag=tag)
            nc.vector.tensor_tensor(
                out=_ap3(y[:], H, ch, ch, 1),
                in0=_ap3(psum_ap, H, ch, ch, 1),
                in1=_ap3(rden[:], H, ch, 1, 0),
                op=OP.mult)
            return y, rden

        # ---- phase B: sg edges -> x1_gene -> xl3 rows, AG2
        ci = 0
        for slot in range(G_PER_CORE):
            xr_slot0 = sb.tile([P, HC1], F32, tag="xrslot1a")
            nc.sync.dma_start(xr_slot0[:], xr1_sg[slot * P:(slot + 1) * P, :])
            xr_slot = sb.tile([P, HC1], F32R, tag="xrslot1")
            nc.scalar.copy(xr_slot[:], xr_slot0[:])
            pm = pse.tile([P, HC1 + H], F32, space="PSUM", tag="pm")
            for k in range(int(g_sched[slot])):
                edge_chunk(ci, sg_srcr_t, sg_dstl_t, tbl_s,
                           xr_slot, att1_sg_t, pm, None,
                           k == 0, k == int(g_sched[slot]) - 1, HC1)
                ci += 1
            y, _ = norm_heads(pm[:], pm[:, HC1:HC1 + H], HC1, "y1g")
            y2 = ev.tile([P, HC1], F32, tag="y2g")
            nc.vector.tensor_tensor(out=y2[:], in0=y[:], in1=bias1_sg_t[:],
                                    op=OP.add)
            x1 = ev.tile([P, HC1], F32, tag="x1g")
            _elu(nc, ev, x1[:], y2[:], HC1)
            xT = _transpose2(nc, sb, psp, ident, x1[:], HC1 // P)
            dense_out(xT, Wl3_t, HC3, bl3_t,
                      dst_ap=agin_3[slot * P:(slot + 1) * P, :], rdt=F32R)
        nc.gpsimd.collective_compute("AllGather", OP.bypass, replica_groups=RG,
                                     ins=[agin_3[:]], outs=[tbl_3[:]])

        # ---- phase C: gs edges -> x1_sample -> xr3/sl3 rows
        ci = 0
        for slot in range(S_PER_CORE):
            xr_slot0 = sb.tile([P, HC1], F32, tag="xrslot1a")
            nc.sync.dma_start(xr_slot0[:], xr1_gs[slot * P:(slot + 1) * P, :])
            xr_slot = sb.tile([P, HC1], F32R, tag="xrslot1")
            nc.scalar.copy(xr_slot[:], xr_slot0[:])
            pm = pse.tile([P, HC1 + H], F32, space="PSUM", tag="pm")
            for k in range(int(s_sched[slot])):
                edge_chunk(ci, gs_srcr_t, gs_dstl_t, tbl_g,
                           xr_slot, att1_gs_t, pm, None,
                           k == 0, k == int(s_sched[slot]) - 1, HC1)
                ci += 1
            y, _ = norm_heads(pm[:], pm[:, HC1:HC1 + H], HC1, "y1s")
            y2 = ev.tile([P, HC1], F32, tag="y2s")
            nc.vector.tensor_tensor(out=y2[:], in0=y[:], in1=bias1_gs_t[:],
                                    op=OP.add)
            y3 = ev.tile([P, HC1], F32, tag="y3s")
            sl1_ap = bass.AP(sl1_sb.tensor,
                             sl1_sb[:, slot * C1:(slot + 1) * C1].offset,
                             [[sl1_sb[:].ap[0][0], P], [0, H], [1, C1]])
            y2v = bass.AP(y2.tensor, y2[:].offset,
                          [[y2[:].ap[0][0], P], [C1, H], [1, C1]])
            y3v = bass.AP(y3.tensor, y3[:].offset,
                          [[y3[:].ap[0][0], P], [C1, H], [1, C1]])
            nc.vector.tensor_tensor(out=y3v, in0=y2v, in1=sl1_ap, op=OP.add)
            x1 = ev.tile([P, HC1], F32, tag="x1s")
            _elu(nc, ev, x1[:], y3[:], HC1)
            xT = _transpose2(nc, sb, psp, ident, x1[:], HC1 // P)
            dense_out(xT, Wr3_t, HC3, br3_t,
                      dst_ap=xr3[slot * P:(slot + 1) * P, :])
            dense_out(xT, sl3_W_t, C3, sl3_b_t,
                      sbuf_dst=sl3_sb[:, slot * C3:(slot + 1) * C3])

        # ---- phase D: gs edges layer 3 -> output
        ci = 0
        for slot in range(S_PER_CORE):
            xr_slot0 = sb.tile([P, HC3], F32, tag="xrslot3a")
            nc.sync.dma_start(xr_slot0[:], xr3[slot * P:(slot + 1) * P, :])
            xr_slot = sb.tile([P, HC3], F32R, tag="xrslot3")
            nc.scalar.copy(xr_slot[:], xr_slot0[:])
            pm = pse.tile([P, HC3], F32, space="PSUM", tag="pm")
            pd = ps1.tile([P, H], F32, space="PSUM", tag="aux")
            for k in range(int(s_sched[slot])):
                edge_chunk(ci, gs_srcr3_t, gs_dstl_t, tbl_3,
                           xr_slot, att3_t, pm, pd,
                           k == 0, k == int(s_sched[slot]) - 1, HC3)
                ci += 1
            # mean over heads of psum_h / den_h == sum_h psum_h * (0.25/den_h)
            den4 = sb.tile([P, H], F32, tag="den")
            nc.vector.tensor_scalar(out=den4[:], in0=pd[:], scalar1=4.0,
                                    scalar2=4e-16, op0=OP.mult, op1=OP.add)
            rden = sb.tile([P, H], F32, tag="rden")
            nc.vector.reciprocal(rden[:], den4[:])
            base = ev.tile([P, C3], F32, tag="based")
            nc.vector.tensor_tensor(out=base[:],
                                    in0=sl3_sb[:, slot * C3:(slot + 1) * C3],
                                    in1=bias3_t[:], op=OP.add)
            accs = [base]
            for h in range(H):
                a = ev.tile([P, C3], F32, tag=f"acc{h}")
                nc.vector.scalar_tensor_tensor(
                    out=a[:], in0=pm[:, h * C3:(h + 1) * C3],
                    scalar=rden[:, h:h + 1], in1=accs[-1][:],
                    op0=OP.mult, op1=OP.add)
                accs.append(a)
            o = ev.tile([P, C3], F32, tag="outt")
            _elu(nc, ev, o[:], accs[-1][:], C3)
            nc.sync.dma_start(out_own[slot * P:(slot + 1) * P, :], o[:])

    nc.compile()
    return nc


# ------------------------------------------------------------------ driver

_CACHE = {}


def kernel(**inputs):
    plan, in_maps = _plan(inputs)
    key = (tuple(plan["g_sched"]), tuple(plan["s_sched"]))
    if key not in _CACHE:
        _CACHE[key] = _build(plan["g_sched"], plan["s_sched"])
    nc = _CACHE[key]
    r = run_bass_kernel_spmd(nc, in_maps, core_ids=list(range(NCORES)))
    out = np.zeros((NS, C3), np.float32)
    s_assign = plan["s_assign"]
    for c in range(NCORES):
        oc = r.results[c]["out_own"]
        for slot in range(S_PER_CORE):
            t = s_assign[slot, c]
            out[t * P:(t + 1) * P] = oc[slot * P:(slot + 1) * P]
    return out



# revision 16
# speedup vs baseline: 2.1100x; 2.1100x over previous
"""BiRGAT (bipartite 2-layer GATv2) Trainium2 kernel, 8-core SPMD. v3.

Destination-tile sharding, bf16 edge pipeline, instruction-count-minimized:
- xl node tables (Wl-transformed, biases folded) in DRAM bf16; one indirect
  DMA per 128-edge chunk gathers source rows (layout [edge, feat]).
- One-hot matrices (edge->dstloc, both orientations) are HOST-precomputed
  bf16 tables streamed per slot on the idle SP DMA queue.
- Per chunk: PE transposes xl into a PSUM zT accumulator and adds the xr
  side via ohT-matmuls against the slot-resident xr tile (no xr gather);
  Act runs a batched Prelu -> g; PE computes alpha = gT @ att_blockdiag;
  Act runs a batched Exp -> ea (bf16); DVE does ONE multiply
  msgs = xl * ea_head; PE scatters [msgs] and [ea] (denominator) with the
  host one-hot as lhsT.
- Softmax normalization at slot evac; conv biases folded into xl tables
  (softmax weights sum to 1); elu's -1 folded into next-layer biases via
  weight column sums.
- Sample-side layer-1 tables computed replicated (no collective); the two
  remaining AllGathers (gene/layer-3 tables) overlap edge compute.
"""
import sys

sys.path.insert(0, "/opt/trn_rl_repo")

import numpy as np
from contextlib import ExitStack

import concourse.bass as bass
import concourse.tile as tile
from concourse import bacc, mybir
from concourse.bass_utils import run_bass_kernel_spmd
from concourse.masks import make_identity

P = 128
NCORES = 8
NS, NG, E = 4096, 20000, 131072
DIN, H, C1, C3 = 256, 4, 64, 128
HC1, HC3 = H * C1, H * C3          # 256, 512
NGP = 20480                        # genes padded to 160 tiles
NST, NGT = NS // P, NGP // P       # 32, 160
S_PER_CORE, G_PER_CORE = NST // NCORES, NGT // NCORES   # 4, 20
SROWS, GROWS = S_PER_CORE * P, G_PER_CORE * P           # 512, 2560

F32 = mybir.dt.float32
BF16 = mybir.dt.float16  # fp16: same speed, 8x tighter mantissa
I32 = mybir.dt.int32
AF = mybir.ActivationFunctionType
OP = mybir.AluOpType

PB1 = 2    # prelu batch (chunks), layer 1
PB3 = 2    # layer 3
EB = 16    # exp batch (chunks)


# ---------------------------------------------------------------- host plan

def _deal_tiles(dst, n_tiles, per_core):
    tcnt = np.bincount(dst // P, minlength=n_tiles)
    chunks = np.maximum((tcnt + P - 1) // P, 1)
    order = np.argsort(-chunks, kind="stable")
    assign = order.reshape(per_core, NCORES)
    sched = chunks[assign].max(axis=1)
    return assign, sched.astype(int)


def _edge_arrays(src, dst, assign, sched, src_row_map, core):
    """Per-core edge data for one relation, slot-grouped, chunk-padded.
    Returns xl_idx [128, nch] i32, oh [128, nch*128] bf16 (edge->dstloc),
    ohT [128, nch*128] bf16 (dstloc->edge)."""
    nch = int(sched.sum())
    xl = np.zeros((nch, P), np.int32)
    oh = np.zeros((P, nch * P), np.float16)
    ohT = np.zeros((P, nch * P), np.float16)
    tile_of = dst // P
    ci = 0
    for slot in range(len(sched)):
        t = assign[slot, core]
        e = np.nonzero(tile_of == t)[0]
        n = len(e)
        nc_s = int(sched[slot])
        sl = np.zeros(nc_s * P, np.int32)
        sl[:n] = src_row_map[src[e]]
        xl[ci:ci + nc_s] = sl.reshape(nc_s, P)
        loc = np.arange(n)
        epart = loc % P
        echunk = ci + loc // P
        dl = (dst[e] % P).astype(np.int64)
        oh[epart, echunk * P + dl] = 1.0
        ohT[dl, echunk * P + epart] = 1.0
        ci += nc_s
    return xl.T.copy(), oh, ohT


def _rhs_tiles(w):
    K, N = w.shape
    return np.ascontiguousarray(
        w.reshape(K // P, P, N).transpose(1, 0, 2)).astype(np.float32)


def _attmat(att, c):
    m = np.zeros((H * c, H), np.float32)
    for h in range(H):
        m[h * c:(h + 1) * c, h] = att[h]
    return m


def _bf16(x):
    return np.ascontiguousarray(np.asarray(x, np.float32)).astype(np.float16)


def _plan(inputs):
    sg_src = np.asarray(inputs["sg_src"]); sg_dst = np.asarray(inputs["sg_dst"])
    gs_src = np.asarray(inputs["gs_src"]); gs_dst = np.asarray(inputs["gs_dst"])

    g_assign, g_sched = _deal_tiles(sg_dst, NGT, G_PER_CORE)
    s_assign, s_sched = _deal_tiles(gs_dst, NST, S_PER_CORE)

    g_owner = np.zeros(NGT, np.int64); g_slot = np.zeros(NGT, np.int64)
    for slot in range(G_PER_CORE):
        for c in range(NCORES):
            g_owner[g_assign[slot, c]] = c
            g_slot[g_assign[slot, c]] = slot
    gid = np.arange(NG)
    grow_tbl = g_owner[gid // P] * GROWS + g_slot[gid // P] * P + gid % P
    srow_tbl = np.arange(NS)   # tbl_s replicated in natural order

    plan = {"g_assign": g_assign, "g_sched": g_sched,
            "s_assign": s_assign, "s_sched": s_sched}

    x_sample = np.asarray(inputs["x_sample"], np.float32)
    x_gene = np.asarray(inputs["x_gene"], np.float32)

    W = {k: np.asarray(inputs[k], np.float32) for k in
         ("Wl1_sg", "Wr1_sg", "Wl1_gs", "Wr1_gs", "Wl3_gs", "Wr3_gs",
          "sl1_W", "sl3_W")}
    b = {k: np.asarray(inputs[k], np.float32) for k in
         ("bl1_sg", "br1_sg", "bias1_sg", "bl1_gs", "br1_gs", "bias1_gs",
          "bl3_gs", "br3_gs", "bias3_gs", "sl1_b", "sl3_b")}

    tile4 = lambda v: np.tile(v, H)
    b_tblS = b["bl1_sg"] + b["bias1_sg"]
    b_xr1sg = b["br1_sg"] - b["bias1_sg"]
    b_tblG = b["bl1_gs"] + b["bias1_gs"] + tile4(b["sl1_b"])
    b_xr1gs = b["br1_gs"] - b["bias1_gs"] - tile4(b["sl1_b"])
    b_tbl3 = b["bl3_gs"] - W["Wl3_gs"].sum(0) + tile4(b["bias3_gs"])
    b_xr3 = b["br3_gs"] - W["Wr3_gs"].sum(0) - tile4(b["bias3_gs"])
    b_sl3 = b["sl3_b"] - W["sl3_W"].sum(0)

    xsT = np.ascontiguousarray(x_sample.T)            # [256, 4096]
    in_maps = []
    for c in range(NCORES):
        s_tiles = s_assign[:, c]
        xs_own = x_sample.reshape(NST, P, DIN)[s_tiles].reshape(SROWS, DIN)
        xsT_own = np.ascontiguousarray(xs_own.T)
        g_tiles = g_assign[:, c]
        xg_own = np.zeros((GROWS, DIN), np.float32)
        for i, t in enumerate(g_tiles):
            lo = t * P
            if lo < NG:
                n = min(P, NG - lo)
                xg_own[i * P:i * P + n] = x_gene[lo:lo + n]
        xgT_own = np.ascontiguousarray(xg_own.T)

        sg_xl, sg_oh, sg_ohT = _edge_arrays(sg_src, sg_dst, g_assign,
                                            g_sched, srow_tbl, c)
        gs_xl, gs_oh, gs_ohT = _edge_arrays(gs_src, gs_dst, s_assign,
                                            s_sched, grow_tbl, c)

        m = {
            "xsT": _bf16(xsT), "xsT_own": _bf16(xsT_own),
            "xgT_own": _bf16(xgT_own),
            "Wl1_sg": _bf16(_rhs_tiles(W["Wl1_sg"])),
            "Wr1_sg": _bf16(_rhs_tiles(W["Wr1_sg"])),
            "Wl1_gs": _bf16(_rhs_tiles(W["Wl1_gs"])),
            "Wr1_gs": _bf16(_rhs_tiles(W["Wr1_gs"])),
            "Wl3": _bf16(_rhs_tiles(W["Wl3_gs"])),
            "Wr3": _bf16(_rhs_tiles(W["Wr3_gs"])),
            "sl1_W": _bf16(_rhs_tiles(W["sl1_W"])),
            "sl3_W": _bf16(_rhs_tiles(W["sl3_W"])),
            "attm1_sg": _bf16(_rhs_tiles(_attmat(np.asarray(inputs["att1_sg"]), C1))),
            "attm1_gs": _bf16(_rhs_tiles(_attmat(np.asarray(inputs["att1_gs"]), C1))),
            "attm3": _bf16(_rhs_tiles(_attmat(np.asarray(inputs["att3_gs"]), C3))),
            "b_tblS": _bf16(b_tblS.reshape(1, -1)),
            "b_xr1sg": _bf16(b_xr1sg.reshape(1, -1)),
            "b_tblG": _bf16(b_tblG.reshape(1, -1)),
            "b_xr1gs": _bf16(b_xr1gs.reshape(1, -1)),
            "b_tbl3": _bf16(b_tbl3.reshape(1, -1)),
            "b_xr3": _bf16(b_xr3.reshape(1, -1)),
            "b_sl3": _bf16(b_sl3.reshape(1, -1)),
            "sg_xl_idx": sg_xl, "sg_oh": sg_oh, "sg_ohT": sg_ohT,
            "gs_xl_idx": gs_xl, "gs_oh": gs_oh, "gs_ohT": gs_ohT,
        }
        in_maps.append(m)
    return plan, in_maps


# ------------------------------------------------------------- device build

def _build(g_sched, s_sched):
    nsg = int(g_sched.sum())
    ngs = int(s_sched.sum())

    nc = bacc.Bacc("TRN2", target_bir_lowering=False, debug=False,
                   num_devices=NCORES)

    ei = lambda name, shape, dt: nc.dram_tensor(name, shape, dt,
                                                kind="ExternalInput")
    xsT = ei("xsT", [DIN, NS], BF16)
    xsT_own = ei("xsT_own", [DIN, SROWS], BF16)
    xgT_own = ei("xgT_own", [DIN, GROWS], BF16)
    Wl1_sg = ei("Wl1_sg", [P, 2, HC1], BF16)
    Wr1_sg = ei("Wr1_sg", [P, 2, HC1], BF16)
    Wl1_gs = ei("Wl1_gs", [P, 2, HC1], BF16)
    Wr1_gs = ei("Wr1_gs", [P, 2, HC1], BF16)
    Wl3 = ei("Wl3", [P, 2, HC3], BF16)
    Wr3 = ei("Wr3", [P, 2, HC3], BF16)
    sl1_W = ei("sl1_W", [P, 2, C1], BF16)
    sl3_W = ei("sl3_W", [P, 2, C3], BF16)
    attm1_sg = ei("attm1_sg", [P, 2, H], BF16)
    attm1_gs = ei("attm1_gs", [P, 2, H], BF16)
    attm3 = ei("attm3", [P, 4, H], BF16)
    b_tblS = ei("b_tblS", [1, HC1], BF16)
    b_xr1sg = ei("b_xr1sg", [1, HC1], BF16)
    b_tblG = ei("b_tblG", [1, HC1], BF16)
    b_xr1gs = ei("b_xr1gs", [1, HC1], BF16)
    b_tbl3 = ei("b_tbl3", [1, HC3], BF16)
    b_xr3 = ei("b_xr3", [1, HC3], BF16)
    b_sl3 = ei("b_sl3", [1, C3], BF16)
    sg_xl_idx = ei("sg_xl_idx", [P, nsg], I32)
    sg_oh = ei("sg_oh", [P, nsg * P], BF16)
    sg_ohT = ei("sg_ohT", [P, nsg * P], BF16)
    gs_xl_idx = ei("gs_xl_idx", [P, ngs], I32)
    gs_oh = ei("gs_oh", [P, ngs * P], BF16)
    gs_ohT = ei("gs_ohT", [P, ngs * P], BF16)

    out_own = nc.dram_tensor("out_own", [SROWS, C3], F32,
                             kind="ExternalOutput")

    tbl_s = nc.dram_tensor("tbl_s", [NS, HC1], BF16)
    agin_g = nc.dram_tensor("agin_g", [GROWS, HC1], BF16)
    tbl_g = nc.dram_tensor("tbl_g", [NGP, HC1], BF16, addr_space="Shared")
    agin_3 = nc.dram_tensor("agin_3", [GROWS, HC3], BF16)
    tbl_3 = nc.dram_tensor("tbl_3", [NGP, HC3], BF16, addr_space="Shared")
    xr1_sg = nc.dram_tensor("xr1_sg", [GROWS, HC1], BF16)
    xr1_gs = nc.dram_tensor("xr1_gs", [SROWS, HC1], BF16)
    xr3 = nc.dram_tensor("xr3", [SROWS, HC3], BF16)

    RG = [list(range(NCORES))]

    with tile.TileContext(nc) as tc, ExitStack() as ctx:
        res = ctx.enter_context(tc.tile_pool(name="res", bufs=1))
        sb = ctx.enter_context(tc.tile_pool(name="sb", bufs=3))
        gp = ctx.enter_context(tc.tile_pool(name="gp", bufs=20))
        ohp = ctx.enter_context(tc.tile_pool(name="ohp", bufs=3))
        ev = ctx.enter_context(tc.tile_pool(name="ev", bufs=2))

        ident_f = res.tile([P, P], F32)
        make_identity(nc, ident_f[:])
        ident = res.tile([P, P], BF16)
        nc.scalar.copy(ident[:], ident_f[:])
        ones1 = res.tile([1, P], BF16)
        nc.vector.memset(ones1[:], 1.0)

        def rload(name, dram, shape, dt):
            t = res.tile(shape, dt, tag=name)
            nc.sync.dma_start(t[:], dram[:])
            return t

        Wl1_sg_t = rload("Wl1_sg", Wl1_sg, [P, 2, HC1], BF16)
        Wr1_sg_t = rload("Wr1_sg", Wr1_sg, [P, 2, HC1], BF16)
        Wl1_gs_t = rload("Wl1_gs", Wl1_gs, [P, 2, HC1], BF16)
        Wr1_gs_t = rload("Wr1_gs", Wr1_gs, [P, 2, HC1], BF16)
        Wl3_t = rload("Wl3", Wl3, [P, 2, HC3], BF16)
        Wr3_t = rload("Wr3", Wr3, [P, 2, HC3], BF16)
        sl1_W_t = rload("sl1_W", sl1_W, [P, 2, C1], BF16)
        sl3_W_t = rload("sl3_W", sl3_W, [P, 2, C3], BF16)
        attm1_sg_t = rload("attm1_sg", attm1_sg, [P, 2, H], BF16)
        attm1_gs_t = rload("attm1_gs", attm1_gs, [P, 2, H], BF16)
        attm3_t = rload("attm3", attm3, [P, 4, H], BF16)
        b_tblS_t = rload("b_tblS", b_tblS, [1, HC1], BF16)
        b_xr1sg_t = rload("b_xr1sg", b_xr1sg, [1, HC1], BF16)
        b_tblG_t = rload("b_tblG", b_tblG, [1, HC1], BF16)
        b_xr1gs_t = rload("b_xr1gs", b_xr1gs, [1, HC1], BF16)
        b_tbl3_t = rload("b_tbl3", b_tbl3, [1, HC3], BF16)
        b_xr3_t = rload("b_xr3", b_xr3, [1, HC3], BF16)
        b_sl3_t = rload("b_sl3", b_sl3, [1, C3], BF16)
        sg_xl_t = rload("sg_xl", sg_xl_idx, [P, nsg], I32)
        gs_xl_t = rload("gs_xl", gs_xl_idx, [P, ngs], I32)

        sl1_sb = res.tile([P, S_PER_CORE * C1], F32)
        sl3_sb = res.tile([P, S_PER_CORE * C3], F32)

        evac_flip = [0]

        # ---------------- phase A: dense node transforms (4-tile groups)
        with tc.tile_pool(name="psA", bufs=3, space="PSUM") as psA:

            def node_group(xT_dram, col0, nt, outs):
                """outs: (W_t, bias_t, n, sink); sink ('dram', tensor, row0)
                or ('sbuf', ap). Processes nt (<=4) consecutive tiles."""
                xT = sb.tile([P, 2, 4 * P], BF16, tag="xTg")
                nc.sync.dma_start(
                    xT[:, :, :nt * P], xT_dram[:, col0:col0 + nt * P]
                    .rearrange("(c p) n -> p c n", p=P))
                for W_t, bias_t, n, sink in outs:
                    pt = psA.tile([P, 4, HC1], F32, tag="ptA")
                    for t in range(nt):
                        for k in range(2):
                            nc.tensor.matmul(
                                pt[:, t, :n],
                                lhsT=xT[:, k, t * P:(t + 1) * P],
                                rhs=W_t[:, k, :n],
                                start=(k == 0),
                                stop=(k == 1 and bias_t is None))
                        if bias_t is not None:
                            nc.tensor.matmul(pt[:, t, :n], lhsT=ones1[:],
                                             rhs=bias_t[:, :n], start=False,
                                             stop=True)
                    if sink[0] == "dram":
                        o = sb.tile([P, 4, HC1], BF16, tag="ntg")
                        if evac_flip[0] % 2 == 0:
                            nc.scalar.copy(o[:, :nt, :n], pt[:, :nt, :n])
                        else:
                            nc.vector.tensor_scalar(out=o[:, :nt, :n],
                                                    in0=pt[:, :nt, :n],
                                                    scalar1=1.0, scalar2=None,
                                                    op0=OP.mult)
                        evac_flip[0] += 1
                        nc.sync.dma_start(
                            sink[1][sink[2]:sink[2] + nt * P, :]
                            .rearrange("(c p) n -> p c n", p=P),
                            o[:, :nt, :n])
                    else:
                        o3 = bass.AP(sink[1].tensor, sink[1].offset,
                                     [[sink[1].ap[0][0], P], [64, nt],
                                      [1, 64]])
                        nc.vector.tensor_scalar(out=o3, in0=pt[:, :nt, :64],
                                                scalar1=1.0, scalar2=None,
                                                op0=OP.mult)

            for g0 in range(0, NST, 4):
                node_group(xsT, g0 * P, 4,
                           [(Wl1_sg_t, b_tblS_t, HC1,
                             ("dram", tbl_s, g0 * P))])
            node_group(xsT_own, 0, 4,
                       [(Wr1_gs_t, b_xr1gs_t, HC1, ("dram", xr1_gs, 0)),
                        (sl1_W_t, None, C1, ("sbuf", sl1_sb[:]))])
            for g0 in range(0, G_PER_CORE, 4):
                node_group(xgT_own, g0 * P, 4,
                           [(Wl1_gs_t, b_tblG_t, HC1,
                             ("dram", agin_g, g0 * P)),
                            (Wr1_sg_t, b_xr1sg_t, HC1,
                             ("dram", xr1_sg, g0 * P))])

        cc1 = nc.gpsimd.collective_compute("AllGather", OP.bypass,
                                           replica_groups=RG,
                                           ins=[agin_g[:]], outs=[tbl_g[:]])
        cc1.engine = mybir.EngineType.SP

        # ---------------- edge phase machinery
        def edge_phase(sched, w, kc, xl_t, oh_dram, ohT_dram,
                       xl_tbl, xr_tbl, attm_t, pb, evac_fn,
                       psz, pa_pool, pso, psd_pool):
            ch = w // H
            ci = 0
            for slot in range(len(sched)):
                nchs = int(sched[slot])
                pso_t = pso.tile([P, w], F32, tag="pso", name="pso_t")
                psd_t = psd_pool.tile([P, H], F32, tag="psd",
                                      name="psd_t")[:]
                xr_sl = sb.tile([P, w], BF16, tag=f"xrs{w}", name="xr_sl")
                nc.sync.dma_start(xr_sl[:],
                                  xr_tbl[slot * P:(slot + 1) * P, :])
                # exp-batch groups of EB chunks
                for e0 in range(0, nchs, EB):
                    en = min(EB, nchs - e0)
                    oh_sl = ohp.tile([P, EB * P], BF16, tag="oh",
                                     name="oh_sl")
                    nc.sync.dma_start(
                        oh_sl[:, :en * P],
                        oh_dram[:, (ci + e0) * P:(ci + e0 + en) * P])
                    ohT_sl = ohp.tile([P, EB * P], BF16, tag="ohT",
                                      name="ohT_sl")
                    nc.sync.dma_start(
                        ohT_sl[:, :en * P],
                        ohT_dram[:, (ci + e0) * P:(ci + e0 + en) * P])
                    pa = pa_pool.tile([P, EB * H], F32, tag="pa", name="pa")
                    gts = []
                    for b0 in range(e0, e0 + en, pb):
                        bn = min(pb, e0 + en - b0)
                        zt = psz.tile([P, pb, w], F32, tag="zt", name="zt")
                        xls = []
                        for b in range(bn):
                            cc = ci + b0 + b
                            xlg = gp.tile([P, w], BF16, tag=f"xlg{w}",
                                          name="xlg")
                            nc.gpsimd.indirect_dma_start(
                                out=xlg[:], out_offset=None, in_=xl_tbl[:],
                                in_offset=bass.IndirectOffsetOnAxis(
                                    ap=xl_t[:, cc:cc + 1], axis=0))
                            xls.append(xlg)
                            ohT_c = ohT_sl[:, (b0 + b - e0) * P:(b0 + b - e0 + 1) * P]
                            for j in range(kc):
                                sl = slice(j * P, (j + 1) * P)
                                nc.tensor.matmul(zt[:, b, sl],
                                                 lhsT=xlg[:, sl],
                                                 rhs=ident[:],
                                                 start=True, stop=False)
                                nc.tensor.matmul(zt[:, b, sl],
                                                 lhsT=xr_sl[:, sl],
                                                 rhs=ohT_c,
                                                 start=False, stop=True)
                        gt = sb.tile([P, pb, w], BF16, tag=f"gt{w}",
                                     name="gt")
                        nc.scalar.activation(gt[:, :bn, :], zt[:, :bn, :],
                                             AF.Prelu, alpha=0.2)
                        for b in range(bn):
                            for j in range(kc):
                                nc.tensor.matmul(
                                    pa[:, (b0 - e0 + b) * H:
                                       (b0 - e0 + b + 1) * H],
                                    lhsT=gt[:, b, j * P:(j + 1) * P],
                                    rhs=attm_t[:, j, :],
                                    start=(j == 0), stop=(j == kc - 1))
                        gts.append((b0, bn, xls))
                    ea = sb.tile([P, EB * H], F32, tag="ea", name="ea")
                    nc.scalar.activation(ea[:, :en * H], pa[:, :en * H],
                                         AF.Exp)
                    ea16 = sb.tile([P, EB * H], BF16, tag="ea16",
                                   name="ea16")
                    nc.vector.tensor_scalar(out=ea16[:, :en * H],
                                            in0=ea[:, :en * H],
                                            scalar1=1.0, scalar2=None,
                                            op0=OP.mult)
                    for b0, bn, xls in gts:
                        for b in range(bn):
                            cc = b0 + b
                            first = cc == 0
                            last = cc == nchs - 1
                            eac = slice((cc - e0) * H, (cc - e0 + 1) * H)
                            msgs = sb.tile([P, w], BF16, tag=f"ms{w}",
                                           name="msgs")
                            ap3m = lambda base, mid, inner: bass.AP(
                                base.tensor, base.offset,
                                [[base.ap[0][0], P], [mid, H], [inner, ch]])
                            nc.vector.tensor_tensor(
                                out=ap3m(msgs[:], ch, 1),
                                in0=ap3m(xls[b][:], ch, 1),
                                in1=ap3m(ea[:, eac], 1, 0), op=OP.mult)
                            oh_c = oh_sl[:, (cc - e0) * P:(cc - e0 + 1) * P]
                            nc.tensor.matmul(pso_t[:, :w], lhsT=oh_c,
                                             rhs=msgs[:],
                                             start=first, stop=last)
                            nc.tensor.matmul(psd_t, lhsT=oh_c,
                                             rhs=ea16[:, eac],
                                             start=first, stop=last)
                ci += nchs
                evac_fn(slot, pso_t, psd_t)

        def norm_y(pso_t, psd_t, w, eps, tag):
            ch = w // H
            dn = ev.tile([P, H], F32, tag="dn")
            nc.vector.tensor_scalar(out=dn[:], in0=psd_t, scalar1=eps,
                                    scalar2=None, op0=OP.add)
            rden = ev.tile([P, H], F32, tag="rden")
            nc.vector.reciprocal(rden[:], dn[:])
            y = ev.tile([P, w], BF16, tag=tag)
            ap3 = lambda base, mid, inner: bass.AP(
                base.tensor, base.offset,
                [[base.ap[0][0], P], [mid, H], [inner, ch]])
            nc.vector.tensor_tensor(out=ap3(y[:], ch, 1),
                                    in0=ap3(pso_t[:], ch, 1),
                                    in1=ap3(rden[:], 1, 0), op=OP.mult)
            return y

        def elu1(y, w, tag):
            m = ev.tile([P, w], BF16, tag=tag + "m")
            nc.vector.tensor_scalar(out=m[:], in0=y if isinstance(y, bass.AP)
                                    else y[:], scalar1=0.0,
                                    scalar2=None, op0=OP.min)
            e = ev.tile([P, w], BF16, tag=tag + "e")
            nc.scalar.activation(e[:], m[:], AF.Exp)
            x1 = ev.tile([P, w], BF16, tag=tag + "x")
            nc.vector.scalar_tensor_tensor(out=x1[:], in0=y if isinstance(
                y, bass.AP) else y[:], scalar=0.0,
                in1=e[:], op0=OP.max, op1=OP.add)
            return x1

        def x1_transpose(x1, px):
            x1T = ev.tile([P, 2, P], BF16, tag="x1T")
            for j in range(2):
                pxt = px.tile([P, P], F32, tag="pxt")
                nc.tensor.matmul(pxt[:], lhsT=x1[:, j * P:(j + 1) * P],
                                 rhs=ident[:], start=True, stop=True)
                if j == 0:
                    nc.scalar.copy(x1T[:, j, :], pxt[:])
                else:
                    nc.vector.tensor_scalar(out=x1T[:, j, :], in0=pxt[:],
                                            scalar1=1.0, scalar2=None,
                                            op0=OP.mult)
            return x1T

        def dense_from_x1T(x1T, W_t, bias_t, n, pt_pool):
            pt = pt_pool.tile([P, HC3], F32, tag="ptE")
            for k in range(2):
                nc.tensor.matmul(pt[:, :n], lhsT=x1T[:, k, :],
                                 rhs=W_t[:, k, :n], start=(k == 0), stop=False)
            nc.tensor.matmul(pt[:, :n], lhsT=ones1[:], rhs=bias_t[:, :n],
                             start=False, stop=True)
            return pt

        # ---------------- phase B: sg edges -> x1_gene -> agin_3
        with tc.tile_pool(name="pszB", bufs=2, space="PSUM") as psz, \
             tc.tile_pool(name="paB", bufs=1, space="PSUM") as pa_pool, \
             tc.tile_pool(name="psoB", bufs=2, space="PSUM") as pso, \
             tc.tile_pool(name="psdB", bufs=1, space="PSUM") as psd, \
             tc.tile_pool(name="pxB", bufs=1, space="PSUM") as px:

            def evac_B(slot, pso_t, psd_t):
                y = norm_y(pso_t, psd_t, HC1, 1e-16, "yB")
                x1 = elu1(y, HC1, "xB")
                x1T = x1_transpose(x1, px)
                pt = dense_from_x1T(x1T, Wl3_t, b_tbl3_t, HC3, px)
                o = ev.tile([P, HC3], BF16, tag="agB")
                nc.scalar.copy(o[:], pt[:])
                nc.sync.dma_start(agin_3[slot * P:(slot + 1) * P, :], o[:])

            edge_phase(g_sched, HC1, 2, sg_xl_t, sg_oh, sg_ohT,
                       tbl_s, xr1_sg, attm1_sg_t, PB1, evac_B,
                       psz, pa_pool, pso, psd)

        cc2 = nc.gpsimd.collective_compute("AllGather", OP.bypass,
                                           replica_groups=RG,
                                           ins=[agin_3[:]], outs=[tbl_3[:]])
        cc2.engine = mybir.EngineType.SP

        # ---------------- phase C: gs edges -> x1_sample -> xr3/sl3
        with tc.tile_pool(name="pszC", bufs=2, space="PSUM") as psz, \
             tc.tile_pool(name="paC", bufs=1, space="PSUM") as pa_pool, \
             tc.tile_pool(name="psoC", bufs=2, space="PSUM") as pso, \
             tc.tile_pool(name="psdC", bufs=1, space="PSUM") as psd, \
             tc.tile_pool(name="pxC", bufs=1, space="PSUM") as px:

            def evac_C(slot, pso_t, psd_t):
                y = norm_y(pso_t, psd_t, HC1, 1e-16, "yC")
                y2 = ev.tile([P, HC1], F32, tag="y2C")
                sl1_ap = bass.AP(sl1_sb.tensor,
                                 sl1_sb[:, slot * C1:(slot + 1) * C1].offset,
                                 [[sl1_sb[:].ap[0][0], P], [0, H], [1, C1]])
                y2v = bass.AP(y2.tensor, y2[:].offset,
                              [[y2[:].ap[0][0], P], [C1, H], [1, C1]])
                yv = bass.AP(y.tensor, y[:].offset,
                             [[y[:].ap[0][0], P], [C1, H], [1, C1]])
                nc.vector.tensor_tensor(out=y2v, in0=yv, in1=sl1_ap,
                                        op=OP.add)
                x1 = elu1(y2, HC1, "xC")
                x1T = x1_transpose(x1, px)
                pt = dense_from_x1T(x1T, Wr3_t, b_xr3_t, HC3, px)
                o = ev.tile([P, HC3], BF16, tag="xrC")
                nc.scalar.copy(o[:], pt[:])
                nc.sync.dma_start(xr3[slot * P:(slot + 1) * P, :], o[:])
                pt2 = dense_from_x1T(x1T, sl3_W_t, b_sl3_t, C3, px)
                nc.vector.tensor_scalar(
                    out=sl3_sb[:, slot * C3:(slot + 1) * C3],
                    in0=pt2[:, :C3], scalar1=1.0, scalar2=None, op0=OP.mult)

            edge_phase(s_sched, HC1, 2, gs_xl_t, gs_oh, gs_ohT,
                       tbl_g, xr1_gs, attm1_gs_t, PB1, evac_C,
                       psz, pa_pool, pso, psd)

        # ---------------- phase D: gs edges layer 3 -> output
        with tc.tile_pool(name="pszD", bufs=2, space="PSUM") as psz, \
             tc.tile_pool(name="paD", bufs=2, space="PSUM") as pa_pool, \
             tc.tile_pool(name="psoD", bufs=1, space="PSUM") as pso, \
             tc.tile_pool(name="psdD", bufs=1, space="PSUM") as psd:

            def evac_D(slot, pso_t, psd_t):
                dn = ev.tile([P, H], F32, tag="dnD")
                nc.vector.tensor_scalar(out=dn[:], in0=psd_t, scalar1=4.0,
                                        scalar2=4e-16, op0=OP.mult, op1=OP.add)
                rden = ev.tile([P, H], F32, tag="rdD")
                nc.vector.reciprocal(rden[:], dn[:])
                cur = sl3_sb[:, slot * C3:(slot + 1) * C3]
                for h in range(H):
                    a = ev.tile([P, C3], F32, tag=f"accD{h}")
                    nc.vector.scalar_tensor_tensor(
                        out=a[:], in0=pso_t[:, h * C3:(h + 1) * C3],
                        scalar=rden[:, h:h + 1], in1=cur,
                        op0=OP.mult, op1=OP.add)
                    cur = a[:]
                m = ev.tile([P, C3], F32, tag="mD")
                nc.vector.tensor_scalar(out=m[:], in0=cur, scalar1=0.0,
                                        scalar2=None, op0=OP.min)
                e = ev.tile([P, C3], F32, tag="eD")
                nc.scalar.activation(e[:], m[:], AF.Exp)
                r = ev.tile([P, C3], F32, tag="rD")
                nc.vector.scalar_tensor_tensor(out=r[:], in0=cur, scalar=0.0,
                                               in1=e[:], op0=OP.max,
                                               op1=OP.add)
                o = ev.tile([P, C3], F32, tag="oD")
                nc.vector.tensor_scalar(out=o[:], in0=r[:], scalar1=-1.0,
                                        scalar2=None, op0=OP.add)
                nc.sync.dma_start(out_own[slot * P:(slot + 1) * P, :], o[:])

            edge_phase(s_sched, HC3, 4, gs_xl_t, gs_oh, gs_ohT,
                       tbl_3, xr3, attm3_t, PB3, evac_D,
                       psz, pa_pool, pso, psd)

    nc.compile()
    return nc


# ------------------------------------------------------------------ driver

_CACHE = {}


def kernel(**inputs):
    plan, in_maps = _plan(inputs)
    key = (tuple(plan["g_sched"]), tuple(plan["s_sched"]))
    if key not in _CACHE:
        _CACHE[key] = _build(plan["g_sched"], plan["s_sched"])
    nc = _CACHE[key]
    r = run_bass_kernel_spmd(nc, in_maps, core_ids=list(range(NCORES)))
    out = np.zeros((NS, C3), np.float32)
    s_assign = plan["s_assign"]
    for c in range(NCORES):
        oc = r.results[c]["out_own"]
        for slot in range(S_PER_CORE):
            t = s_assign[slot, c]
            out[t * P:(t + 1) * P] = oc[slot * P:(slot + 1) * P]
    return out
